# revision 28
# baseline (speedup 1.0000x reference)
"""Trainium2 Bass kernel for BiasedMHA (B=4, N=1024, C=1024, H=16, D=64).

Sharding: 8 cores = 4 batches x 2 head-halves (8 heads each).
Each core computes q/k/v projections for its head slice, biased softmax
attention, and its partial o_proj.  Host sums the two partials per batch
and adds the (bo + bv @ Wo.T) constant.

v3: the attn_bias add is moved OFF the tensor engine entirely.  Host
ships expB = exp(attn_bias) in fp16; on-chip the ACT engine computes
eraw = exp(q.k) from PSUM and the DVE multiplies eraw * expB (fp16
2x-mode), which equals exp(q.k + bias).  The q/k projection bias rows
are folded into the PSUM->SBUF evacuation (tensor_scalar add of a
per-partition bias column).  Softmax normalization uses a gpsimd
partition_broadcast of the fp32 reciprocal instead of the ones-matmul
broadcast.  Net: the PE runs only real GEMM columns (projections,
scores, PV, o_proj).

Layouts (host-prepped, contraction-on-partitions):
  xT    [C, N]  fp16 : x[b].T
  wqT   [C, 512] fp16: Wq[rows,:].T * scale (softmax scale folded here)
  wkT/wvT [C, 512] fp16
  woT   [512, C] fp16: Wo[:, cols].T
  expB  [8, N, N] fp16: exp(attn_bias[b, heads]).transpose(0,2,1) ([h, m, n])
  bqc/bkc [128, 4] fp32: q/k bias per (j-in-tile partition, jt)
  madd  [128, 8] fp32: additive mask (-1e30 where attn_mask==0), m-tiled

Attention per head: S^T[m, n] = k^T(d,m).T @ q^T(d,n) (head pairs
row-packed, K=64 at array rows 0-63/64-127), exp on ACT straight from
PSUM (mask as per-partition bias; no max-subtraction -- scores O(+-4)),
DVE mult by expB tile.  P@V consumes the product; v carries a ones
column per head so PV row 64 is the softmax denominator.  Normalize:
reciprocal_approx_fast on the PSUM denominator row, partition_broadcast
(gpsimd) to 64 rows, DVE multiply into attT[j, n] fp16, then o_proj;
partials returned fp16 and summed on host in fp32.
"""

import os
import sys

if "/opt/trn_rl_repo" not in sys.path:
    sys.path.insert(0, "/opt/trn_rl_repo")

from contextlib import ExitStack

import numpy as np

# debug toggles (temporary)
DBG_EVAC_TS = os.environ.get("K_EVAC", "ts") == "ts"     # tensor_scalar evac
DBG_NORM_PB = os.environ.get("K_NORM", "pb") == "pb"     # partition_broadcast
DBG_EXP_PSUM = os.environ.get("K_EXPPSUM", "0") == "1"   # exp in-place in PSUM
DBG_OUT_D2D = os.environ.get("K_OUT", "q") == "d2d"      # output via DIRECT2D

B, N, C, H = 4, 1024, 1024, 16
D = C // H            # 64
HL = H // 2           # 8 local heads per core
JL = HL * D           # 512 local head dims
NT = N // 128         # 8 seq tiles
CT = C // 128         # 8 contraction tiles
SCALE = D ** (-0.5)

_prog_cache = {}


def build_program():
    import concourse.tile as tile
    from concourse import bacc, mybir
    f32 = mybir.dt.float32
    f16 = mybir.dt.float16

    nc = bacc.Bacc("TRN2", target_bir_lowering=False, debug=False,
                   enable_asserts=False, num_devices=8)

    xT = nc.dram_tensor("xT", [C, N], f16, kind="ExternalInput").ap()
    wqT = nc.dram_tensor("wqT", [C, JL], f16, kind="ExternalInput").ap()
    wkT = nc.dram_tensor("wkT", [C, JL], f16, kind="ExternalInput").ap()
    wvT = nc.dram_tensor("wvT", [C, JL], f16, kind="ExternalInput").ap()
    woT = nc.dram_tensor("woT", [JL, C], f16, kind="ExternalInput").ap()
    bqc = nc.dram_tensor("bqc", [128, 4], f32, kind="ExternalInput").ap()
    bkc = nc.dram_tensor("bkc", [128, 4], f32, kind="ExternalInput").ap()
    expB = nc.dram_tensor("expB", [HL, N, N], f16, kind="ExternalInput").ap()
    madd = nc.dram_tensor("madd", [128, NT], f32, kind="ExternalInput").ap()
    outp = nc.dram_tensor("outp", [N, C], f16, kind="ExternalOutput").ap()

    Exp = mybir.ActivationFunctionType.Exp
    mult_op = mybir.AluOpType.mult
    add_op = mybir.AluOpType.add
    from concourse.tile_rust import add_dep_helper
    first_evac = [None]

    from concourse import library_config

    with tile.TileContext(nc) as tc, ExitStack() as ctx:
        # partition_broadcast is a custom gpsimd op: its firmware library
        # must be loaded before first use
        if DBG_NORM_PB:
            nc.gpsimd.load_library(library_config.attn)
        # ---- pools ----
        resident = ctx.enter_context(tc.tile_pool(name="resident", bufs=1))
        biaspool = ctx.enter_context(tc.tile_pool(name="bias", bufs=20))
        rawpool = ctx.enter_context(tc.tile_pool(name="raws", bufs=6))
        exppool = ctx.enter_context(tc.tile_pool(name="exps", bufs=8))
        outpool = ctx.enter_context(tc.tile_pool(name="outs", bufs=3))
        smallpool = ctx.enter_context(tc.tile_pool(name="small", bufs=3))
        ps_main = ctx.enter_context(
            tc.tile_pool(name="ps_main", bufs=2, space="PSUM"))
        ps_pv = ctx.enter_context(
            tc.tile_pool(name="ps_pv", bufs=2, space="PSUM"))

        # ---- resident tiles ----
        xts = resident.tile([128, CT, N], f16)          # xT tiled on c
        wq_sb = resident.tile([128, CT, JL], f16)       # wqT tiled on c
        wk_sb = resident.tile([128, CT, JL], f16)       # wkT tiled on c
        wv_sb = resident.tile([128, CT, JL], f16)       # wvT tiled on c
        wo_sb = resident.tile([128, 4, C], f16)         # woT tiled on j
        xview = xT.rearrange("(ct p) n -> p ct n", p=128)
        wqview = wqT.rearrange("(ct p) j -> p ct j", p=128)
        # startup DMAs chunked per c-tile and interleaved (x0, wq0, x1,
        # wq1, ...) so the first q-projection matmuls can start as soon
        # as their tiles land instead of waiting for one bulk transfer.
        last = None
        for ct in range(CT):
            nc.sync.dma_start(xts[:, ct, :], xview[:, ct, :])
            last = nc.sync.dma_start(wq_sb[:, ct, :], wqview[:, ct, :])
        bulk = [
            nc.sync.dma_start(wk_sb[:],
                              wkT.rearrange("(ct p) j -> p ct j", p=128)),
            nc.sync.dma_start(wv_sb[:],
                              wvT.rearrange("(ct p) j -> p ct j", p=128)),
            nc.sync.dma_start(wo_sb[:],
                              woT.rearrange("(kt p) c -> p kt c", p=128)),
        ]
        for b in bulk:
            add_dep_helper(b.ins, last.ins, reason="startup DMA priority")

        madd_sb = resident.tile([128, NT], f32)
        nc.sync.dma_start(madd_sb[:], madd)
        bq_sb = resident.tile([128, 4], f32)
        nc.sync.dma_start(bq_sb[:], bqc)
        bk_sb = resident.tile([128, 4], f32)
        nc.sync.dma_start(bk_sb[:], bkc)

        ones_f32 = resident.tile([128, 1], f32)
        nc.vector.memset(ones_f32[:], 1.0)
        ones_row = resident.tile([1, N], f16)
        nc.vector.tensor_copy(
            ones_row[:], ones_f32[0:1, 0:1].to_broadcast([1, N]))

        qT_sb = resident.tile([128, 4, N], f16)         # [j-tile, n]
        kT_sb = resident.tile([128, 4, N], f16)
        v_sb = resident.tile([128, NT, HL * (D + 1)], f16)  # [m-tile, h*65]
        # attT as one tile per pair: tile-granularity dependencies would
        # otherwise make every o_proj matmul wait on the LAST pair's
        # normalize write
        attT_p = [resident.tile([128, N], f16, name=f"attT_p{i}")
                  for i in range(4)]

        # ones columns of v (softmax denominator trick)
        for mt in range(NT):
            v4 = v_sb[:, mt, :].rearrange("p (h c) -> p h c", c=D + 1)
            nc.vector.tensor_copy(
                v4[:, :, D:D + 1],
                ones_f32[:, 0:1, None].to_broadcast([128, HL, 1]))

        # ---- PE warm-up: spin the HAM activity window while DMAs land ----
        warm_sb = resident.tile([128, 512], f16)
        nc.vector.memset(warm_sb[:], 0.0)
        warm_ps = ps_pv.tile([128, 512], f32, tag="pv", name="warm")
        for i in range(8):
            nc.tensor.matmul(warm_ps[:], warm_sb[:, 0:128], warm_sb[:],
                             start=True, stop=True)

        # ---- phase 1: projections ----
        # q/k transposed: out[j-tile, n] = sum_c wT[c, j] * xT[c, n];
        # the per-j bias is added during PSUM evacuation (tensor_scalar).
        for (wsb, bcol, dest) in ((wq_sb, bq_sb, qT_sb), (wk_sb, bk_sb, kT_sb)):
            for jt in range(4):
                ps = ps_main.tile([128, N], f32, tag="mm")
                for ct in range(CT):
                    w = wsb[:, ct, jt * 128:(jt + 1) * 128]
                    for nh in range(2):
                        nc.tensor.matmul(
                            ps[:, nh * 512:(nh + 1) * 512],
                            w[:],
                            xts[:, ct, nh * 512:(nh + 1) * 512],
                            start=(ct == 0), stop=(ct == CT - 1))
                if DBG_EVAC_TS:
                    ev = nc.vector.tensor_scalar(
                        out=dest[:, jt, :], in0=ps[:],
                        scalar1=bcol[:, jt:jt + 1], scalar2=None, op0=add_op)
                else:
                    ev = nc.vector.tensor_copy(dest[:, jt, :], ps[:])
                if first_evac[0] is None:
                    first_evac[0] = ev

        # v normal layout: out[m-tile, j] = sum_c xT[c, m] * wvT[c, j]
        for mt in range(NT):
            ps = ps_main.tile([128, N], f32, tag="mm")
            psv = ps[:, 0:JL]
            for ct in range(CT):
                nc.tensor.matmul(
                    psv,
                    xts[:, ct, mt * 128:(mt + 1) * 128],
                    wv_sb[:, ct, :],
                    start=(ct == 0), stop=(ct == CT - 1))
            v4 = v_sb[:, mt, :].rearrange("p (h c) -> p h c", c=D + 1)
            nc.vector.tensor_copy(
                v4[:, :, 0:D],
                psv.rearrange("p (h c) -> p h c", c=D))

        # ---- phase 2: attention, one head pair at a time ----
        # Software-pipelined (lag-1): PV for step g-1 is emitted alongside
        # scores for step g, so the PE never head-of-line waits on the
        # ACT exp + DVE expB-multiply of the current step.
        def emit_scores(hp, mt):
            hA, hB = 2 * hp, 2 * hp + 1
            ebt = [None, None]
            for hi, h in enumerate((hA, hB)):
                b_ = biaspool.tile([128, N], f16, tag="bias",
                                   name=f"bias_{hp}_{mt}_{hi}")
                bdma = nc.gpsimd.dma_start(
                    b_[:], expB[h, mt * 128:(mt + 1) * 128, :])
                if hp == 0 and first_evac[0] is not None:
                    add_dep_helper(bdma.ins, first_evac[0].ins,
                                   reason="bias prefetch behind startup loads")
                ebt[hi] = b_
            sps = []
            for hi in range(2):
                sp = ps_main.tile([128, N], f32, tag="mm",
                                  name=f"s_{hp}_{mt}_{hi}")
                sps.append(sp)
            # S matmuls in alternating row groups (rows 0-63 / 64-127)
            for nh in range(2):
                sl = slice(nh * 512, (nh + 1) * 512)
                for hi in range(2):
                    base = hi * 64
                    nc.tensor.matmul(
                        sps[hi][:, sl],
                        kT_sb[base:base + 64, hp, mt * 128:(mt + 1) * 128],
                        qT_sb[base:base + 64, hp, sl],
                        start=True, stop=True)
            out = []
            for hi in range(2):
                if DBG_EXP_PSUM:
                    # exp overwrites the score PSUM in place: saves the
                    # raw-exp SBUF round-trip (the attention phase is SBUF-
                    # bandwidth-bound); the DVE multiply reads PSUM instead
                    nc.scalar.activation(sps[hi][:], sps[hi][:], Exp,
                                         bias=madd_sb[:, mt:mt + 1])
                    er_src = sps[hi]
                else:
                    er = rawpool.tile([128, N], f16, tag="raw",
                                      name=f"raw_{hp}_{mt}_{hi}")
                    nc.scalar.activation(er[:], sps[hi][:], Exp,
                                         bias=madd_sb[:, mt:mt + 1])
                    er_src = er
                et = exppool.tile([128, N], f16, tag="exp",
                                  name=f"exp_{hp}_{mt}_{hi}")
                for nh in range(2):
                    sl = slice(nh * 512, (nh + 1) * 512)
                    nc.vector.tensor_tensor(
                        et[:, sl], er_src[:, sl], ebt[hi][:, sl], mult_op)
                out.append(et)
            return out

        def emit_pv(hp, mt, pv, ets):
            for hi, h in enumerate((2 * hp, 2 * hp + 1)):
                vx = v_sb[:, mt, h * 65:(h + 1) * 65]
                for nh in range(2):
                    nc.tensor.matmul(
                        pv[hi][0:65, nh * 512:(nh + 1) * 512],
                        vx,
                        ets[hi][:, nh * 512:(nh + 1) * 512],
                        start=(mt == 0), stop=(mt == NT - 1))

        def emit_normalize(hp, pv, last=False):
            if not last:
                # Heads' chains interleaved so DVE (den/recip) and Pool
                # (broadcast) pipeline across hi.
                dens, recips, bcs = [], [], []
                for hi in range(2):
                    den = smallpool.tile([1, N], f32, tag="den",
                                         name=f"den{hp}{hi}")
                    nc.vector.tensor_copy(den[:], pv[hi][64:65, :])
                    dens.append(den)
                for hi in range(2):
                    recip32 = smallpool.tile([1, N], f32, tag="recip32",
                                             name=f"r32_{hp}{hi}")
                    nc.vector.reciprocal_approx_fast(
                        out=recip32[:], in_=dens[hi][:])
                    recips.append(recip32)
                for hi in range(2):
                    bc32 = smallpool.tile([64, N], f32, tag="bc32",
                                          name=f"bc32_{hp}{hi}")
                    nc.gpsimd.partition_broadcast(bc32[:], recips[hi][:])
                    bcs.append(bc32)
                for hi in range(2):
                    nc.vector.tensor_tensor(
                        attT_p[hp][hi * 64:hi * 64 + 64, :],
                        pv[hi][0:64, :], bcs[hi][:], mult_op)
            else:
                # Last pair gates the o_proj phase: run the whole chain per
                # (nh, hi) quadrant so the first attT quadrant (and with it
                # the first o_proj kt-last matmuls) unblocks in ~half the
                # chain latency.  The extra instructions only land where the
                # PE is idle anyway.
                for nh in range(2):
                    sl = slice(nh * 512, (nh + 1) * 512)
                    for hi in range(2):
                        den = smallpool.tile([1, 512], f32, tag="den",
                                             name=f"dq{hp}{hi}{nh}")
                        nc.vector.tensor_copy(den[:], pv[hi][64:65, sl])
                        recip32 = smallpool.tile([1, 512], f32, tag="recip32",
                                                 name=f"rq_{hp}{hi}{nh}")
                        nc.vector.reciprocal_approx_fast(
                            out=recip32[:], in_=den[:])
                        bc32 = smallpool.tile([64, 512], f32, tag="bc32",
                                              name=f"bq_{hp}{hi}{nh}")
                        nc.gpsimd.partition_broadcast(bc32[:], recip32[:])
                        nc.vector.tensor_tensor(
                            attT_p[hp][hi * 64:hi * 64 + 64, sl],
                            pv[hi][0:64, sl], bc32[:], mult_op)

        pv_by_pair = {}
        pending = []
        for g in range(4 * NT):
            hp, mt = divmod(g, NT)
            if mt == 0:
                pv_by_pair[hp] = [
                    ps_pv.tile([128, N], f32, tag="pv", name=f"pv_{hp}_{i}")
                    for i in range(2)]
            ets = emit_scores(hp, mt)
            pending.append((hp, mt, ets))
            if g >= 1:
                php, pmt, pets = pending.pop(0)
                emit_pv(php, pmt, pv_by_pair[php], pets)
                if pmt == NT - 1:
                    emit_normalize(php, pv_by_pair[php])
        php, pmt, pets = pending.pop(0)
        emit_pv(php, pmt, pv_by_pair[php], pets)
        emit_normalize(php, pv_by_pair[php], last=True)

        # ---- phase 3: o_proj partial ----
        for nt in range(NT):
            ps = ps_main.tile([128, N], f32, tag="mm")
            for ch in range(2):
                for kt in range(4):
                    nc.tensor.matmul(
                        ps[:, ch * 512:(ch + 1) * 512],
                        attT_p[kt][:, nt * 128:(nt + 1) * 128],
                        wo_sb[:, kt, ch * 512:(ch + 1) * 512],
                        start=(kt == 0), stop=(kt == 3))
            ot = outpool.tile([128, N], f16, tag="out")
            nc.vector.tensor_copy(ot[:], ps[:])
            if DBG_OUT_D2D:
                # DIRECT2D (Pool) write-out: faster than the DMA queues and
                # the Pool engine is idle during phase 3
                nc.gpsimd.dma_start(outp[nt * 128:(nt + 1) * 128, :], ot[:])
            else:
                nc.sync.dma_start(outp[nt * 128:(nt + 1) * 128, :], ot[:])

    nc.compile()
    return nc


def get_program():
    if "nc" not in _prog_cache:
        _prog_cache["nc"] = build_program()
    return _prog_cache["nc"]


def make_in_maps(x, attn_bias, attn_mask, Wq, bq, Wk, bk, Wv, bv, Wo, bo):
    """Host-side shard + layout prep.  Returns (in_maps, const) where
    const[c_out] = bo + bv @ Wo.T must be added to the gathered output."""
    x = np.asarray(x, np.float32)
    attn_bias = np.asarray(attn_bias, np.float32)
    attn_mask = np.asarray(attn_mask)
    Wq = np.asarray(Wq, np.float32)
    Wk = np.asarray(Wk, np.float32)
    Wv = np.asarray(Wv, np.float32)
    Wo = np.asarray(Wo, np.float32)
    bq = np.asarray(bq, np.float32)
    bk = np.asarray(bk, np.float32)
    bv = np.asarray(bv, np.float32)
    bo = np.asarray(bo, np.float32)

    const = bo + bv @ Wo.T

    xTs = [np.ascontiguousarray(x[b].T).astype(np.float16) for b in range(B)]
    madds = []
    for b in range(B):
        ma = np.where(attn_mask[b] == 0, np.float32(-1e30), np.float32(0.0))
        madds.append(np.ascontiguousarray(ma.reshape(NT, 128).T))

    in_maps = []
    for core in range(8):
        b, half = divmod(core, 2)
        rows = slice(half * JL, (half + 1) * JL)
        wqT = np.ascontiguousarray(
            (Wq[rows, :] * np.float32(SCALE)).T).astype(np.float16)
        wkT = np.ascontiguousarray(Wk[rows, :].T).astype(np.float16)
        wvT = np.ascontiguousarray(Wv[rows, :].T).astype(np.float16)
        woT = np.ascontiguousarray(Wo[:, rows].T).astype(np.float16)
        bqcol = np.ascontiguousarray(
            (bq[rows] * np.float32(SCALE)).reshape(4, 128).T).astype(np.float32)
        bkcol = np.ascontiguousarray(
            bk[rows].reshape(4, 128).T).astype(np.float32)
        eb = np.ascontiguousarray(np.exp(
            attn_bias[b, half * HL:(half + 1) * HL]).transpose(0, 2, 1)
        ).astype(np.float16)
        in_maps.append({
            "xT": xTs[b], "wqT": wqT, "wkT": wkT, "wvT": wvT, "woT": woT,
            "bqc": bqcol, "bkc": bkcol, "expB": eb,
            "madd": madds[b],
        })
    return in_maps, const


def gather(results, const):
    out = np.empty((B, N, C), np.float32)
    for b in range(B):
        out[b] = results[2 * b]["outp"].astype(np.float32) \
            + results[2 * b + 1]["outp"].astype(np.float32) \
            + const[None, :]
    return out


def kernel(**inputs):
    from concourse.bass_utils import run_bass_kernel_spmd
    nc = get_program()
    in_maps, const = make_in_maps(**inputs)
    res = run_bass_kernel_spmd(nc, in_maps, core_ids=list(range(8)))
    return gather(res.results, const)
